# revision 18
# baseline (speedup 1.0000x reference)
"""Dynamic per-pixel 3x3 filtering on 8 Trainium2 NeuronCores.

out[b,c,y,x] = sum_{ki,kj} img[b,c,y+ki-1,x+kj-1] * kernels[b,c,ki*3+kj,y,x]
(zero padding outside the image).

Sharding: pure data parallel, one batch sample per core (B=8, 8 cores).

Per-core layout: each channel's [512, 512] image plane is viewed as
[128 partitions, 4 blocks, 512 cols] (row r = block*128 + partition).
Row-shifted variants (y-1 / y+1) are built ON-CHIP via the idle
TensorE: the plane is transposed in 128x128 chunks into a zero-padded
"rows in the free dim" tile, then transposed back with a +-1 free-dim
offset (a partition shift is impossible on the lockstep compute engines,
SBUF->SBUF partition-shift DMAs serialize onto one SDMA engine, and
re-reading shifted rows from HBM costs 2.5 MB/channel of the bottleneck
HBM bandwidth). ScalarE evacuates PSUM. Column shifts are free-dim AP
offsets.

All 17 elementwise passes (9 mult + 8 accumulate) run on the Vector
engine: concurrent GPSIMD tensor_tensor work contends with DVE for the
shared SBUF port (measured 2.5x DVE slowdown), so a tap split across
engines loses. DMA issue is split across both HWDGE sequencers (SP for
image/shift traffic, ACT for kernel-tile loads and stores) because a
single sequencer serializes on per-DMA descriptor generation.
"""

from contextlib import ExitStack

import numpy as np

import concourse.bacc as bacc
import concourse.mybir as mybir
import concourse.tile as tile
from concourse import masks
from concourse.bass_utils import run_bass_kernel_spmd

C, H, W = 3, 512, 512
KK = 9
NCORES = 8
P = 128
NB = H // P          # 4 row blocks per channel
FW = NB * W          # 2048 free-dim width of a channel mega-tile
F32 = mybir.dt.float32

# Taps: t = ki*3 + kj; row shift = ki-1 (top/mid/bot), col shift = kj-1.
# mid taps first (no shift-DMA dependency); first tap must be dx=0 (full write).
TAP_ORDER = [4, 3, 5, 1, 0, 2, 7, 6, 8]


def _r3(ap):
    """[128, FW] -> [128, NB, W] block view of a channel mega-tile."""
    return ap.rearrange("p (b x) -> p b x", x=W)


def _emit(nc, tc, ctx):
    img = nc.dram_tensor("img", (C, H, W), F32, kind="ExternalInput").ap()
    ker = nc.dram_tensor("kernels", (C, KK, H, W), F32, kind="ExternalInput").ap()
    out = nc.dram_tensor("out", (C, H, W), F32, kind="ExternalOutput").ap()

    v_pool = ctx.enter_context(tc.tile_pool(name="v", bufs=2))
    k_pool = ctx.enter_context(tc.tile_pool(name="k", bufs=13))
    acc_pool = ctx.enter_context(tc.tile_pool(name="acc", bufs=2))
    tmp_pool = ctx.enter_context(tc.tile_pool(name="tmp", bufs=1))
    t_pool = ctx.enter_context(tc.tile_pool(name="tp", bufs=2))
    ps_pool = ctx.enter_context(tc.tile_pool(name="ps", bufs=8, space="PSUM"))
    id_pool = ctx.enter_context(tc.tile_pool(name="ident", bufs=1))

    ident = id_pool.tile([P, P], F32, tag="ident")
    masks.make_identity(nc, ident[:, :])
    NXC = W // P          # 4 col-chunks of 128
    TS = H + 2            # 514: transposed row axis incl. zero pads

    for c in range(C):
        # img rows for this channel: mid[p, b*W + x] = img[c, b*128 + p, x]
        mid = v_pool.tile([P, FW], F32, tag="mid")
        nc.sync.dma_start(
            _r3(mid[:, :]), img[c].rearrange("(b p) x -> p b x", p=P)
        )
        kts = {}
        for t in TAP_ORDER:
            kt = k_pool.tile([P, FW], F32, tag="kt")
            nc.scalar.dma_start(
                _r3(kt[:, :]), ker[c, t].rearrange("(b p) x -> p b x", p=P)
            )
            kts[t] = kt

        # Transposed plane: T[xp, xc*TS + 1 + r] = img[c, r, xc*128 + xp],
        # with zero columns at slot 0 (row -1) and slot 513 (row 512).
        T = t_pool.tile([P, NXC * TS], F32, tag="T")
        for xc in range(NXC):
            nc.scalar.memzero(T[:, xc * TS : xc * TS + 1])
            nc.scalar.memzero(T[:, xc * TS + TS - 1 : xc * TS + TS])
        for b in range(NB):
            for xc in range(NXC):
                ps = ps_pool.tile([P, P], F32, tag="ps")
                nc.tensor.transpose(
                    ps[:, :], mid[:, b * W + xc * P : b * W + (xc + 1) * P],
                    ident[:, :],
                )
                nc.scalar.copy(
                    T[:, xc * TS + 1 + b * P : xc * TS + 1 + (b + 1) * P],
                    ps[:, :],
                )
        # top[q, b*512 + xc*128 + xp] = img row (128b + q - 1) -> transpose
        # back from T with free offset 0; bot with free offset 2.
        top = v_pool.tile([P, FW], F32, tag="top")
        bot = v_pool.tile([P, FW], F32, tag="bot")
        for dst, off in ((top, 0), (bot, 2)):
            for b in range(NB):
                for xc in range(NXC):
                    ps = ps_pool.tile([P, P], F32, tag="ps")
                    nc.tensor.transpose(
                        ps[:, :],
                        T[:, xc * TS + off + b * P : xc * TS + off + (b + 1) * P],
                        ident[:, :],
                    )
                    nc.scalar.copy(
                        dst[:, b * W + xc * P : b * W + (xc + 1) * P], ps[:, :]
                    )


        acc = acc_pool.tile([P, FW], F32, tag="acc")
        tmp = tmp_pool.tile([P, FW], F32, tag="tmp")
        vs = [top, mid, bot]
        eng = nc.vector
        first = True
        for t in TAP_ORDER:
            ki, kj = divmod(t, 3)
            v, dx = vs[ki], kj - 1
            if dx == 0:
                if first:
                    eng.tensor_mul(acc[:, :], v[:, :], kts[t][:, :])
                else:
                    eng.tensor_mul(tmp[:, :], v[:, :], kts[t][:, :])
                    eng.tensor_add(acc[:, :], acc[:, :], tmp[:, :])
            else:
                a3, v3, k3 = _r3(acc[:, :]), _r3(v[:, :]), _r3(kts[t][:, :])
                tsl = _r3(tmp[:, :])[:, :, 0 : W - 1]
                if dx < 0:
                    asl, vsl, ksl = a3[:, :, 1:W], v3[:, :, 0 : W - 1], k3[:, :, 1:W]
                else:
                    asl, vsl, ksl = a3[:, :, 0 : W - 1], v3[:, :, 1:W], k3[:, :, 0 : W - 1]
                eng.tensor_mul(tsl, vsl, ksl)
                eng.tensor_add(asl, asl, tsl)
            first = False

        # Store via SWDGE (gpsimd) — a third DMA queue, so the store's
        # wait-for-compute never blocks the HWDGE load rings.
        nc.gpsimd.dma_start(
            out[c].rearrange("(b p) x -> p b x", p=P), _r3(acc[:, :])
        )


_NC_CACHE = []


def _build():
    nc = bacc.Bacc(
        "TRN2",
        target_bir_lowering=False,
        debug=False,
        enable_asserts=True,
        num_devices=1,
    )
    with tile.TileContext(nc) as tc:
        with ExitStack() as ctx:
            _emit(nc, tc, ctx)
    nc.compile()
    return nc


def kernel(img, kernels):
    """img: [8, 3, 512, 512] f32; kernels: [8, 3, 9, 512, 512] f32.
    Returns [8, 3, 512, 512] f32."""
    first_call = not _NC_CACHE
    if first_call:
        _NC_CACHE.append(_build())
    nc = _NC_CACHE[0]
    img = np.asarray(img, dtype=np.float32)
    kernels = np.asarray(kernels, dtype=np.float32)
    in_maps = [
        {
            "img": np.ascontiguousarray(img[b]),
            "kernels": np.ascontiguousarray(kernels[b]),
        }
        for b in range(NCORES)
    ]
    if first_call:
        # Warm-up execution: the very first run after a fresh NEFF
        # compile/load was observed to occasionally return stale output.
        run_bass_kernel_spmd(nc, in_maps, core_ids=list(range(NCORES)))
    res = run_bass_kernel_spmd(nc, in_maps, core_ids=list(range(NCORES)))
    return np.stack([res.results[b]["out"] for b in range(NCORES)], axis=0)
